# revision 20
# baseline (speedup 1.0000x reference)
"""Trainium2 Bass kernel for ConstrainedProbabilityMatrixFactorization.

rating = uw @ iw.T + ub + ib.T + bias + (fb_values . E[fb_indices]) @ iw.T
       = ue_aug @ rhs_aug
  with ue_aug  = [uw + offset | ub + bias | 1]   [BU, 66]
       rhs_aug = [iw.T ; ones ; ib.T]            [66, BI]

Sharding: the 1024-user batch is split across 8 NeuronCores (128 users
per core). No collectives.

The dominant cost is SWDGE (Q7) descriptor generation for the feedback
segment-gather (~8ns/index, serialized per Q7 core pair). Key tricks:
  * 4 SWDGE queues: dma_gather(queue_num=q) runs on Q7 core pair
    (2q, 2q+1), so gathers on queues 0-3 generate descriptors
    CONCURRENTLY -> ~4x on the bottleneck.
  * 2 waves per queue (8 gathers total) so the DVE weighted-reduce of
    wave A overlaps wave B's descriptor generation.
  * gather from a PAIRED bf16 view of item_rating_effect_weight
    [25000, 128]: index = row//2 fits int16, 256B rows (elem%256==0),
    half the HBM traffic, and DVE runs at 2x on 16-bit. Row parity is
    resolved by host-built interleaved weights w2[p, 2s+parity] (other
    half-slot weight 0) folded into the DVE multiply.
  * gp slot layout is wave-major so each wave's multiply+reduce is ONE
    contiguous DVE op pair (2 mult + 2 reduce total).
  * bf16 matmuls (PE full rate vs fp32 1/4 rate), fp32 PSUM accum.
  * user/item batch rows are host-prepped (ue0 = [uw|ub+bias|1],
    rhs_aug = [iw.T;ones;ib.T]) -- the on-device work is the
    segment-reduce + the rating matmul.

Per-core program:
  1. dma in: idx tile, then w2/ue0/rhs/ident.
  2. 8 dma_gathers (4 queues x 2 waves) -> gp [128, 50, 128] bf16.
  3. per wave: prod = w2 . gp (DVE bf16), oh = reduce_s(prod) f32.
  4. ue0[:, :64] += ohA + ohB; PE transpose ue0 -> ueT bf16 [66, 128].
  5. 8 matmuls [66,128]^T @ [66,512] bf16 -> PSUM f32 -> SBUF -> DMA.
"""

import numpy as np
import ml_dtypes

N_USERS = 100000
N_ITEMS = 50000
NPAIR = N_ITEMS // 2       # 25000 paired rows; index fits int16
D = 64
D2 = 2 * D                 # 128: paired row width
BU = 1024
BI = 4096
L = 50
NCORES = 8
UB = BU // NCORES          # 128 users per core
P = 128
K = D + 2                  # 66: augmented contraction dim
NBANK = 8                  # output column blocks of 512
NQ = 4                     # SWDGE queues

# Gather plan: (queue, orig slot range, gp slot start). Queue-0 gathers
# execute SYNCHRONOUSLY on the GpSimd engine stream (blocking later
# instruction dispatch), while queues 1-3 retire instantly and generate
# descriptors asynchronously on their Q7 core pairs. So: dispatch all
# queue-1/2/3 gathers first (their pairs start at t0 and chew through
# their work back-to-back), queue-0's gathers last (cores 0/1 idle-pop
# the async ones, then work concurrently with the pairs).
# The SDMA doorbell fires once per gather at the END of its descriptor
# generation, so wave A is split into 2 sub-gathers per queue: the
# first sub's drain overlaps the second sub's generation instead of
# the whole wave's ~1MB draining serially after the last doorbell.
# Wave A = gp slots [0,32), wave B = [32,50); the DVE multiply+fold of
# wave A overlaps wave B's descriptor generation.
# At most 8 gather instructions: more overflows the GpSimd broadcast
# FIFO and serializes dispatch. At most 8 slots per gather: 9+ slots
# (73+ descriptors per SDMA lane) overflows the SWDGE descriptor ring
# and hangs in await_space. Queue 0 runs ~1.4x slower per slot than the
# async pairs, so it gets fewer slots (9 vs 13-14).
GATHERS = [
    # (queue, orig_lo, orig_hi, gp_lo)
    (1, 0, 8, 0),
    (2, 14, 22, 8),
    (3, 28, 36, 16),
    (1, 8, 14, 30),
    (2, 22, 28, 36),
    (3, 36, 41, 42),
    (0, 41, 47, 24),
    (0, 47, 50, 47),
]
NSA = 30                   # wave A slots
NSB = L - NSA              # wave B slots
IDXCOLS = 8 * L            # idx tile cols: 8 per slot (128 idx / 16)

_cached = {}


def _build_program():
    import concourse.bacc as bacc
    import concourse.bass as bass
    import concourse.mybir as mybir
    import concourse.tile as tile

    f32 = mybir.dt.float32
    bf16 = mybir.dt.bfloat16
    i16 = mybir.dt.int16

    # Bacc (not raw Bass): its compile() legalizes sync waits for TRN2.
    nc = bacc.Bacc(num_swdge_queues=NQ)

    idx = nc.dram_tensor("idx", [P, IDXCOLS], i16, kind="ExternalInput")
    w2 = nc.dram_tensor("w2", [P, 2 * L], bf16, kind="ExternalInput")
    ue0 = nc.dram_tensor("ue0", [P, K], f32, kind="ExternalInput")
    ereP = nc.dram_tensor("ereP", [NPAIR, D2], bf16, kind="ExternalInput")
    rhs_in = nc.dram_tensor("rhs", [K, BI], bf16, kind="ExternalInput")
    ident_in = nc.dram_tensor("ident_in", [P, P], f32, kind="ExternalInput")
    rating = nc.dram_tensor("rating", [UB, BI], f32, kind="ExternalOutput")

    with tile.TileContext(nc) as tc:
        with (
            tc.tile_pool(name="sb", bufs=1) as sb,
            tc.tile_pool(name="sb_out", bufs=8) as sb_out,
            tc.tile_pool(name="ps_ue", bufs=1, space="PSUM") as ps_ue,
            tc.tile_pool(name="ps_mm", bufs=7, space="PSUM") as ps_mm,
        ):
            # --- index tile, then the gathers immediately ---
            i_s = sb.tile([P, IDXCOLS], i16)
            nc.sync.dma_start(out=i_s[:], in_=idx[:])

            gp = sb.tile([P, L * D2], bf16)  # [128, 50, 128] paired rows
            for q, olo, ohi, glo in GATHERS:
                nw = ohi - olo
                ghi = glo + nw
                nc.gpsimd.dma_gather(
                    out_ap=gp[:, glo * D2 : ghi * D2].rearrange(
                        "p (l e) -> p l e", e=D2
                    ),
                    in_ap=ereP[:],
                    idxs_ap=i_s[:, 8 * glo : 8 * ghi],
                    num_idxs=P * nw,
                    num_idxs_reg=P * nw,
                    elem_size=D2,
                    single_packet=True,
                    queue_num=q,
                )

            # --- small early load: w2 (25KB). Everything else streams via
            # SWDGE after the gathers so early SDMA traffic doesn't slow
            # the ~6us Q7 IRAM library load that gates the first gather.
            w2_s = sb.tile([P, 2 * L], bf16)
            nc.sync.dma_start(out=w2_s[:], in_=w2[:])

            # --- bulk loads (small + rhs 0.54MB) on the HWDGE path.
            ue = sb.tile([P, K], f32)
            nc.sync.dma_start(out=ue[:], in_=ue0[:])
            ident = sb.tile([P, P], f32)
            nc.sync.dma_start(out=ident[:], in_=ident_in[:])
            rhs = sb.tile([K, BI], bf16)
            nc.sync.dma_start(out=rhs[:], in_=rhs_in[:])

            # --- expand w2 [P, 100] -> w2f [P, 100*64] on the idle DVE
            # during the gather phase: the multiplies then run elementwise
            # in the DVE's 2x 16-bit mode (a stride-0 broadcast operand
            # would force 1x).
            w2f_s = sb.tile([P, L * D2], bf16)
            nc.vector.tensor_copy(
                out=w2f_s[:].rearrange("p (s d) -> p s d", d=D),
                in_=w2_s[:].to_broadcast([P, 2 * L, D]),
            )

            # --- offset: per-wave elementwise multiply + contiguous bf16
            # fold + strided reduce.
            spans = ((0, NSA), (NSA, L))
            offs = []
            for w, (slo, shi) in enumerate(spans):
                sz = (shi - slo) * D2
                prod = sb.tile([P, sz], bf16, tag=f"prod{w}")
                nc.vector.tensor_tensor(
                    out=prod[:],
                    in0=gp[:, slo * D2 : shi * D2],
                    in1=w2f_s[:, slo * D2 : shi * D2],
                    op=mybir.AluOpType.mult,
                )
                while sz % (2 * D) == 0 and sz > 18 * D:
                    half = sz // 2
                    nc.vector.tensor_tensor(
                        out=prod[:, 0:half],
                        in0=prod[:, 0:half],
                        in1=prod[:, half:sz],
                        op=mybir.AluOpType.add,
                    )
                    sz = half
                oh = sb.tile([P, D], f32, tag=f"offs{w}")
                nc.vector.reduce_sum(
                    out=oh[:],
                    in_=prod[:, 0:sz].rearrange("p (s d) -> p d s", d=D),
                    axis=mybir.AxisListType.X,
                )
                offs.append(oh)
            # ue[:, :D] += offsA + offsB
            nc.vector.tensor_tensor(
                out=offs[0][:], in0=offs[0][:], in1=offs[1][:],
                op=mybir.AluOpType.add,
            )
            nc.vector.tensor_tensor(
                out=ue[:, 0:D], in0=ue[:, 0:D], in1=offs[0][:],
                op=mybir.AluOpType.add,
            )

            # --- transpose ue -> ueT [66, 128] bf16 ---
            ueT_p = ps_ue.tile([K, P], f32, space="PSUM")
            nc.tensor.transpose(out=ueT_p[:], in_=ue[:], identity=ident[:])
            ueT = sb.tile([K, P], bf16)
            nc.scalar.copy(out=ueT[:], in_=ueT_p[:])

            # --- main matmuls + output ---
            for n in range(NBANK):
                mm = ps_mm.tile([P, 512], f32, space="PSUM", tag="mm")
                nc.tensor.matmul(
                    out=mm[:],
                    lhsT=ueT[:],
                    rhs=rhs[:, n * 512 : (n + 1) * 512],
                    start=True,
                    stop=True,
                )
                ot = sb_out.tile([P, 512], f32, tag="ot")
                # alternate copy engines so PSUM->SBUF keeps pace with PE
                if n % 2 == 0:
                    nc.scalar.copy(out=ot[:], in_=mm[:])
                else:
                    nc.vector.tensor_copy(out=ot[:], in_=mm[:])
                nc.sync.dma_start(
                    out=rating[:, n * 512 : (n + 1) * 512], in_=ot[:]
                )

    nc.finalize()
    return nc


def _get_program():
    if "nc" not in _cached:
        _cached["nc"] = _build_program()
    return _cached["nc"]


# tile[p, c] = flat[c*16 + p%16]: dma_gather index interleave,
# replicated across the 8 groups of 16 partitions.
def _wrap_idx(flat):
    n16 = len(flat) // 16
    sidx = np.arange(n16)[None, :] * 16 + (np.arange(P) % 16)[:, None]
    return flat[sidx]


_IDENT = np.eye(P, dtype=np.float32)
BF16 = ml_dtypes.bfloat16


def _prep_inputs(inputs):
    user_ids = np.asarray(inputs["user_ids"]).astype(np.int64)
    item_ids = np.asarray(inputs["item_ids"]).astype(np.int64)
    fb_indices = np.asarray(inputs["fb_indices"]).astype(np.int64)
    fb_values = np.asarray(inputs["fb_values"]).astype(np.float32)
    uw = np.asarray(inputs["user_weight"], dtype=np.float32)
    ub = np.asarray(inputs["user_bias"], dtype=np.float32).reshape(N_USERS, 1)
    iw = np.asarray(inputs["item_weight"], dtype=np.float32)
    ib = np.asarray(inputs["item_bias"], dtype=np.float32).reshape(N_ITEMS, 1)
    ire = np.ascontiguousarray(
        np.asarray(inputs["item_rating_effect_weight"], dtype=np.float32)
    )
    bias = float(np.asarray(inputs["bias"], dtype=np.float32).reshape(-1)[0])

    # item batch: order known host-side; device streams it contiguously
    rhs = np.empty((K, BI), dtype=BF16)
    rhs[0:D] = iw[item_ids].T.astype(BF16)
    rhs[D] = 1.0
    rhs[D + 1] = ib[item_ids, 0].astype(BF16)

    ereP = ire.reshape(NPAIR, D2).astype(BF16)             # paired view

    # original slot -> gp slot permutation (wave-major layout)
    perm = np.empty(L, dtype=np.int64)
    for q, olo, ohi, glo in GATHERS:
        perm[olo:ohi] = np.arange(glo, glo + (ohi - olo))

    in_maps = []
    for c in range(NCORES):
        sl = slice(c * UB, (c + 1) * UB)
        fbi_c = fb_indices[sl]                 # [128, 50]
        fbv_c = fb_values[sl]
        pair_idx = (fbi_c // 2).astype(np.int16)
        parity = (fbi_c & 1).astype(np.int64)

        idx_tile = np.empty((P, IDXCOLS), dtype=np.int16)
        for q, olo, ohi, glo in GATHERS:
            flat = pair_idx[:, olo:ohi].T.reshape(-1)  # [l*128+p]
            idx_tile[:, 8 * glo : 8 * (glo + ohi - olo)] = _wrap_idx(flat)

        # w2[p, 2*gpslot + parity] = fbv[p, l]; other half-slot weight 0.
        # Expanded to full width on-device (device DVE broadcast copy).
        w2v = np.zeros((P, 2 * L), dtype=np.float32)
        rows = np.repeat(np.arange(P), L)
        cols = (2 * perm[None, :] + parity).reshape(-1)
        w2v[rows, cols] = fbv_c.reshape(-1)

        ue0 = np.empty((P, K), dtype=np.float32)
        uids = user_ids[sl]
        ue0[:, 0:D] = uw[uids]
        ue0[:, D] = ub[uids, 0] + bias
        ue0[:, D + 1] = 1.0

        in_maps.append(
            {
                "idx": idx_tile,
                "w2": w2v.astype(BF16),
                "ue0": ue0,
                "ereP": ereP,
                "rhs": rhs,
                "ident_in": _IDENT,
            }
        )
    return in_maps


def run(inputs, trace=False):
    """Returns (output [1024, 4096] f32, BassKernelResults)."""
    from concourse import bass_utils

    nc = _get_program()
    in_maps = _prep_inputs(inputs)
    res = bass_utils.run_bass_kernel_spmd(
        nc, in_maps, core_ids=list(range(NCORES)), trace=trace
    )
    out = np.concatenate([res.results[c]["rating"] for c in range(NCORES)], axis=0)
    return out, res


def kernel(**inputs) -> np.ndarray:
    out, _ = run(inputs, trace=False)
    return out


# revision 23
# speedup vs baseline: 1.0480x; 1.0480x over previous
"""Trainium2 Bass kernel for ConstrainedProbabilityMatrixFactorization.

rating = uw @ iw.T + ub + ib.T + bias + (fb_values . E[fb_indices]) @ iw.T
       = ue_aug @ rhs_aug
  with ue_aug  = [uw + offset | ub + bias | 1]   [BU, 66]
       rhs_aug = [iw.T ; ones ; ib.T]            [66, BI]

Sharding: the 1024-user batch is split across 8 NeuronCores (128 users
per core). No collectives.

The dominant cost is SWDGE (Q7) descriptor generation for the feedback
segment-gather (~8ns/index, serialized per Q7 core pair). Key tricks:
  * 4 SWDGE queues: dma_gather(queue_num=q) runs on Q7 core pair
    (2q, 2q+1), so gathers on queues 0-3 generate descriptors
    CONCURRENTLY -> ~4x on the bottleneck.
  * 2 waves per queue (8 gathers total) so the DVE weighted-reduce of
    wave A overlaps wave B's descriptor generation.
  * gather from a PAIRED bf16 view of item_rating_effect_weight
    [25000, 128]: index = row//2 fits int16, 256B rows (elem%256==0),
    half the HBM traffic, and DVE runs at 2x on 16-bit. Row parity is
    resolved by host-built interleaved weights w2[p, 2s+parity] (other
    half-slot weight 0) folded into the DVE multiply.
  * gp slot layout is wave-major so each wave's multiply+reduce is ONE
    contiguous DVE op pair (2 mult + 2 reduce total).
  * bf16 matmuls (PE full rate vs fp32 1/4 rate), fp32 PSUM accum.
  * user/item batch rows are host-prepped (ue0 = [uw|ub+bias|1],
    rhs_aug = [iw.T;ones;ib.T]) -- the on-device work is the
    segment-reduce + the rating matmul.

Per-core program:
  1. dma in: idx tile, then w2/ue0/rhs/ident.
  2. 8 dma_gathers (4 queues x 2 waves) -> gp [128, 50, 128] bf16.
  3. per wave: prod = w2 . gp (DVE bf16), oh = reduce_s(prod) f32.
  4. ue0[:, :64] += ohA + ohB; PE transpose ue0 -> ueT bf16 [66, 128].
  5. 8 matmuls [66,128]^T @ [66,512] bf16 -> PSUM f32 -> SBUF -> DMA.
"""

import numpy as np
import ml_dtypes

N_USERS = 100000
N_ITEMS = 50000
NPAIR = N_ITEMS // 2       # 25000 paired rows; index fits int16
D = 64
D2 = 2 * D                 # 128: paired row width
BU = 1024
BI = 4096
L = 50
NCORES = 8
UB = BU // NCORES          # 128 users per core
P = 128
K = D + 2                  # 66: augmented contraction dim
NBANK = 8                  # output column blocks of 512
NQ = 4                     # SWDGE queues

# Gather plan: (queue, orig slot range, gp slot start). Queue-0 gathers
# execute SYNCHRONOUSLY on the GpSimd engine stream (blocking later
# instruction dispatch), while queues 1-3 retire instantly and generate
# descriptors asynchronously on their Q7 core pairs. So: dispatch all
# queue-1/2/3 gathers first (their pairs start at t0 and chew through
# their work back-to-back), queue-0's gathers last (cores 0/1 idle-pop
# the async ones, then work concurrently with the pairs).
# The SDMA doorbell fires once per gather at the END of its descriptor
# generation, so wave A is split into 2 sub-gathers per queue: the
# first sub's drain overlaps the second sub's generation instead of
# the whole wave's ~1MB draining serially after the last doorbell.
# Wave A = gp slots [0,32), wave B = [32,50); the DVE multiply+fold of
# wave A overlaps wave B's descriptor generation.
# At most 8 gather instructions: more overflows the GpSimd broadcast
# FIFO and serializes dispatch. At most 8 slots per gather: 9+ slots
# (73+ descriptors per SDMA lane) overflows the SWDGE descriptor ring
# and hangs in await_space. Queue 0 runs ~1.4x slower per slot than the
# async pairs, so it gets fewer slots (9 vs 13-14).
GATHERS = [
    # (queue, orig_lo, orig_hi, gp_lo)
    (1, 0, 8, 0),
    (2, 14, 22, 8),
    (3, 28, 36, 16),
    (1, 8, 14, 30),
    (2, 22, 28, 36),
    (3, 36, 41, 42),
    (0, 41, 47, 24),
    (0, 47, 50, 47),
]
NSA = 30                   # wave A slots
NSB = L - NSA              # wave B slots
IDXCOLS = 8 * L            # idx tile cols: 8 per slot (128 idx / 16)

_cached = {}


def _build_program():
    import concourse.bacc as bacc
    import concourse.bass as bass
    import concourse.mybir as mybir
    import concourse.tile as tile

    f32 = mybir.dt.float32
    bf16 = mybir.dt.bfloat16
    i16 = mybir.dt.int16

    # Bacc (not raw Bass): its compile() legalizes sync waits for TRN2.
    nc = bacc.Bacc(num_swdge_queues=NQ)

    idx = nc.dram_tensor("idx", [P, IDXCOLS], i16, kind="ExternalInput")
    w2 = nc.dram_tensor("w2", [P, 2 * L], bf16, kind="ExternalInput")
    ue0 = nc.dram_tensor("ue0", [P, K], f32, kind="ExternalInput")
    ereP = nc.dram_tensor("ereP", [NPAIR, D2], bf16, kind="ExternalInput")
    rhs_in = nc.dram_tensor("rhs", [K, BI], bf16, kind="ExternalInput")
    ident_in = nc.dram_tensor("ident_in", [P, P], f32, kind="ExternalInput")
    rating = nc.dram_tensor("rating", [UB, BI], f32, kind="ExternalOutput")

    with tile.TileContext(nc) as tc:
        with (
            tc.tile_pool(name="sb", bufs=1) as sb,
            tc.tile_pool(name="sb_out", bufs=8) as sb_out,
            tc.tile_pool(name="ps_ue", bufs=1, space="PSUM") as ps_ue,
            tc.tile_pool(name="ps_mm", bufs=7, space="PSUM") as ps_mm,
        ):
            # --- index tile, then the gathers immediately ---
            i_s = sb.tile([P, IDXCOLS], i16)
            nc.sync.dma_start(out=i_s[:], in_=idx[:])

            gp = sb.tile([P, L * D2], bf16)  # [128, 50, 128] paired rows
            for q, olo, ohi, glo in GATHERS:
                nw = ohi - olo
                ghi = glo + nw
                nc.gpsimd.dma_gather(
                    out_ap=gp[:, glo * D2 : ghi * D2].rearrange(
                        "p (l e) -> p l e", e=D2
                    ),
                    in_ap=ereP[:],
                    idxs_ap=i_s[:, 8 * glo : 8 * ghi],
                    num_idxs=P * nw,
                    num_idxs_reg=P * nw,
                    elem_size=D2,
                    single_packet=True,
                    queue_num=q,
                )

            # --- small early load: w2 (25KB). Everything else streams via
            # SWDGE after the gathers so early SDMA traffic doesn't slow
            # the ~6us Q7 IRAM library load that gates the first gather.
            w2_s = sb.tile([P, 2 * L], bf16)
            nc.sync.dma_start(out=w2_s[:], in_=w2[:])

            # --- small loads on the HWDGE path (cheap); rhs (0.54MB) via
            # SWDGE *after* the gathers so its transfer doesn't slow the
            # ~6us Q7 IRAM library load that gates the first gather. It
            # dispatches once queue-0's sync gathers retire and lands well
            # before the matmuls need it.
            ue = sb.tile([P, K], f32)
            nc.sync.dma_start(out=ue[:], in_=ue0[:])
            ident = sb.tile([P, P], f32)
            nc.sync.dma_start(out=ident[:], in_=ident_in[:])
            rhs = sb.tile([K, BI], bf16)
            nc.gpsimd.dma_start(out=rhs[:], in_=rhs_in[:])

            # --- expand w2 [P, 100] -> w2f [P, 100*64] on the idle DVE
            # during the gather phase: the multiplies then run elementwise
            # in the DVE's 2x 16-bit mode (a stride-0 broadcast operand
            # would force 1x).
            w2f_s = sb.tile([P, L * D2], bf16)
            nc.vector.tensor_copy(
                out=w2f_s[:].rearrange("p (s d) -> p s d", d=D),
                in_=w2_s[:].to_broadcast([P, 2 * L, D]),
            )

            # --- offset: per-wave elementwise multiply + contiguous bf16
            # fold + strided reduce. Both waves share ONE prod tile: the
            # WAR hazard forces the Tile scheduler to sequence the whole
            # wave-A chain before multB, instead of interleaving a wave-B
            # data wait into the middle of wave A's folds.
            spans = ((0, NSA), (NSA, L))
            prod = sb.tile([P, NSA * D2], bf16)
            offs = []
            for w, (slo, shi) in enumerate(spans):
                sz = (shi - slo) * D2
                nc.vector.tensor_tensor(
                    out=prod[:, 0:sz],
                    in0=gp[:, slo * D2 : shi * D2],
                    in1=w2f_s[:, slo * D2 : shi * D2],
                    op=mybir.AluOpType.mult,
                )
                while sz % (2 * D) == 0 and sz > 18 * D:
                    half = sz // 2
                    nc.vector.tensor_tensor(
                        out=prod[:, 0:half],
                        in0=prod[:, 0:half],
                        in1=prod[:, half:sz],
                        op=mybir.AluOpType.add,
                    )
                    sz = half
                oh = sb.tile([P, D], f32, tag=f"offs{w}")
                nc.vector.reduce_sum(
                    out=oh[:],
                    in_=prod[:, 0:sz].rearrange("p (s d) -> p d s", d=D),
                    axis=mybir.AxisListType.X,
                )
                offs.append(oh)
            # ue[:, :D] += offsA + offsB
            nc.vector.tensor_tensor(
                out=offs[0][:], in0=offs[0][:], in1=offs[1][:],
                op=mybir.AluOpType.add,
            )
            nc.vector.tensor_tensor(
                out=ue[:, 0:D], in0=ue[:, 0:D], in1=offs[0][:],
                op=mybir.AluOpType.add,
            )

            # --- transpose ue -> ueT [66, 128] bf16 ---
            ueT_p = ps_ue.tile([K, P], f32, space="PSUM")
            nc.tensor.transpose(out=ueT_p[:], in_=ue[:], identity=ident[:])
            ueT = sb.tile([K, P], bf16)
            nc.scalar.copy(out=ueT[:], in_=ueT_p[:])

            # --- main matmuls + output ---
            for n in range(NBANK):
                mm = ps_mm.tile([P, 512], f32, space="PSUM", tag="mm")
                nc.tensor.matmul(
                    out=mm[:],
                    lhsT=ueT[:],
                    rhs=rhs[:, n * 512 : (n + 1) * 512],
                    start=True,
                    stop=True,
                )
                ot = sb_out.tile([P, 512], f32, tag="ot")
                # alternate copy + DMA-issue engines so PSUM->SBUF->HBM
                # keeps pace with the PE
                if n % 2 == 0:
                    nc.scalar.copy(out=ot[:], in_=mm[:])
                    nc.sync.dma_start(
                        out=rating[:, n * 512 : (n + 1) * 512], in_=ot[:]
                    )
                else:
                    nc.vector.tensor_copy(out=ot[:], in_=mm[:])
                    nc.scalar.dma_start(
                        out=rating[:, n * 512 : (n + 1) * 512], in_=ot[:]
                    )

    nc.finalize()
    return nc


def _get_program():
    if "nc" not in _cached:
        _cached["nc"] = _build_program()
    return _cached["nc"]


# tile[p, c] = flat[c*16 + p%16]: dma_gather index interleave,
# replicated across the 8 groups of 16 partitions.
def _wrap_idx(flat):
    n16 = len(flat) // 16
    sidx = np.arange(n16)[None, :] * 16 + (np.arange(P) % 16)[:, None]
    return flat[sidx]


_IDENT = np.eye(P, dtype=np.float32)
BF16 = ml_dtypes.bfloat16


def _prep_inputs(inputs):
    user_ids = np.asarray(inputs["user_ids"]).astype(np.int64)
    item_ids = np.asarray(inputs["item_ids"]).astype(np.int64)
    fb_indices = np.asarray(inputs["fb_indices"]).astype(np.int64)
    fb_values = np.asarray(inputs["fb_values"]).astype(np.float32)
    uw = np.asarray(inputs["user_weight"], dtype=np.float32)
    ub = np.asarray(inputs["user_bias"], dtype=np.float32).reshape(N_USERS, 1)
    iw = np.asarray(inputs["item_weight"], dtype=np.float32)
    ib = np.asarray(inputs["item_bias"], dtype=np.float32).reshape(N_ITEMS, 1)
    ire = np.ascontiguousarray(
        np.asarray(inputs["item_rating_effect_weight"], dtype=np.float32)
    )
    bias = float(np.asarray(inputs["bias"], dtype=np.float32).reshape(-1)[0])

    # item batch: order known host-side; device streams it contiguously
    rhs = np.empty((K, BI), dtype=BF16)
    rhs[0:D] = iw[item_ids].T.astype(BF16)
    rhs[D] = 1.0
    rhs[D + 1] = ib[item_ids, 0].astype(BF16)

    ereP = ire.reshape(NPAIR, D2).astype(BF16)             # paired view

    # original slot -> gp slot permutation (wave-major layout)
    perm = np.empty(L, dtype=np.int64)
    for q, olo, ohi, glo in GATHERS:
        perm[olo:ohi] = np.arange(glo, glo + (ohi - olo))

    in_maps = []
    for c in range(NCORES):
        sl = slice(c * UB, (c + 1) * UB)
        fbi_c = fb_indices[sl]                 # [128, 50]
        fbv_c = fb_values[sl]
        pair_idx = (fbi_c // 2).astype(np.int16)
        parity = (fbi_c & 1).astype(np.int64)

        idx_tile = np.empty((P, IDXCOLS), dtype=np.int16)
        for q, olo, ohi, glo in GATHERS:
            flat = pair_idx[:, olo:ohi].T.reshape(-1)  # [l*128+p]
            idx_tile[:, 8 * glo : 8 * (glo + ohi - olo)] = _wrap_idx(flat)

        # w2[p, 2*gpslot + parity] = fbv[p, l]; other half-slot weight 0.
        # Expanded to full width on-device (device DVE broadcast copy).
        w2v = np.zeros((P, 2 * L), dtype=np.float32)
        rows = np.repeat(np.arange(P), L)
        cols = (2 * perm[None, :] + parity).reshape(-1)
        w2v[rows, cols] = fbv_c.reshape(-1)

        ue0 = np.empty((P, K), dtype=np.float32)
        uids = user_ids[sl]
        ue0[:, 0:D] = uw[uids]
        ue0[:, D] = ub[uids, 0] + bias
        ue0[:, D + 1] = 1.0

        in_maps.append(
            {
                "idx": idx_tile,
                "w2": w2v.astype(BF16),
                "ue0": ue0,
                "ereP": ereP,
                "rhs": rhs,
                "ident_in": _IDENT,
            }
        )
    return in_maps


def run(inputs, trace=False):
    """Returns (output [1024, 4096] f32, BassKernelResults)."""
    from concourse import bass_utils

    nc = _get_program()
    in_maps = _prep_inputs(inputs)
    res = bass_utils.run_bass_kernel_spmd(
        nc, in_maps, core_ids=list(range(NCORES)), trace=trace
    )
    out = np.concatenate([res.results[c]["rating"] for c in range(NCORES)], axis=0)
    return out, res


def kernel(**inputs) -> np.ndarray:
    out, _ = run(inputs, trace=False)
    return out
